# revision 24
# baseline (speedup 1.0000x reference)
"""BERT-CRF NER on Trainium2: emissions (matmul+sigmoid) and Viterbi forward
recursion on device, data-parallel over batch across 8 NeuronCores; host does
only the O(B*S*L) backtrack.

Key structural facts exploited (valid for contiguous masks, which is what the
reference's setup_inputs produces; a full host fallback covers anything else):
  - The CLS/SEP compaction is a pure row-selection, so it commutes with the
    linear projection: compact(x) @ W == gather rows of (x @ W). With a
    contiguous mask the gather is just a shift by one token.
  - Emission values at masked Viterbi steps never influence the decoded path
    (masked steps freeze the score and use identity backpointers), so the
    device can run the UNMASKED recurrence; score history beyond a sample's
    last valid step is simply never read by the host backtrack.
  - Backpointers are reconstructed on host from the device's score history:
    bp_t[c] = argmax_p(score_{t-1}[p] + T[p,c]) with bitwise-identical f32
    adds, so the reconstruction matches an on-device argmax.
  - token_features/W are fed in fp16: the quantization flips a handful of
    near-tie path elements (measured ~7/32768 vs the f32 reference, far under
    the 2e-2 gate) and halves transfer, DMA and matmul-stream time, and
    enables xbar DMA-transpose (unsupported for fp32).

Shapes (hardcoded per problem spec): B=128, S=256, H=768, L=24, 8 cores.
"""

import numpy as np

B, S, H, L = 128, 256, 768, 24
N_CORES = 8
BS = B // N_CORES          # 16 samples per core
R = BS * S                 # 4096 token rows per core
NK = H // 128              # 6 contraction chunks
NG = R // 512              # 8 column groups for the emissions matmul

_DEVICE_STATE = {}


# ---------------------------------------------------------------- device ----

def _build_nc():
    import concourse.mybir as mybir
    from concourse.bass import ts
    from concourse import bacc, tile

    f32 = mybir.dt.float32
    f16 = mybir.dt.float16
    nc = bacc.Bacc()
    x = nc.dram_tensor("x", [R, H], f16, kind="ExternalInput")
    w = nc.dram_tensor("w", [H, L], f16, kind="ExternalInput")
    bt = nc.dram_tensor("bt", [L, 1], f32, kind="ExternalInput")
    gmat = nc.dram_tensor("gmat", [128, 128], f32, kind="ExternalInput")
    m0 = nc.dram_tensor("m0", [128, L], f32, kind="ExternalInput")
    tblk = nc.dram_tensor("tblk", [128, (L // 8) * L], f32, kind="ExternalInput")
    stq = nc.dram_tensor("stq", [128, L // 8], f32, kind="ExternalInput")
    emat = nc.dram_tensor("emat", [BS, 128], f32, kind="ExternalInput")
    cbsel = nc.dram_tensor("cbsel", [8, 128], f32, kind="ExternalInput")
    tb8 = nc.dram_tensor("tb8", [8, (L // 8) * L], f32, kind="ExternalInput")
    hist = nc.dram_tensor("hist", [128, (L // 8) * S], f32, kind="ExternalOutput")
    emq2 = nc.dram_tensor("emq2", [BS, L * S], f32, kind="ExternalOutput")

    with tile.TileContext(nc) as tc:
        with (
            tc.tile_pool(name="const", bufs=1) as cpool,
            tc.tile_pool(name="xin", bufs=6) as xpool,
            tc.tile_pool(name="xt", bufs=2) as xtpool,
            tc.tile_pool(name="emt", bufs=3) as empool,
            tc.tile_pool(name="vit", bufs=1) as vpool,
            tc.tile_pool(name="tp", bufs=2, space="PSUM") as tppool,
            tc.tile_pool(name="mm", bufs=2, space="PSUM") as mmpool,
            tc.tile_pool(name="sp", bufs=2, space="PSUM") as sppool,
            tc.tile_pool(name="dram", bufs=1, space="DRAM") as dpool,
        ):
            from concourse import masks
            ident = cpool.tile([128, 128], f16, tag="ident")
            masks.make_identity(nc, ident[:, :])
            wk = []
            for k in range(NK):
                wt = cpool.tile([128, L], f16, tag=f"w{k}")
                nc.sync.dma_start(out=wt[:, :], in_=w[ts(k, 128), :])
                wk.append(wt)
            bsb = cpool.tile([L, 1], f32, tag="bias")
            nc.sync.dma_start(out=bsb[:, :], in_=bt[:, :])

            em_dramT = dpool.tile([L, R], f32)

            # ---- emissions: em.T[L, R] = sigmoid(W.T @ x.T + b) ----
            # Per 4-row-tile group: load x fp16, transpose 128x128 chunks on
            # TensorE, copy PSUM->SBUF (split across Vector/Scalar), then 6
            # accumulating matmuls with a 512-wide moving operand.
            for g in range(NG):  # 8 groups of 4 row-tiles (512 rows)
                # xT layout [128, (k:6) x (512 cols)] = x[g*512:(g+1)*512].T
                xT = xtpool.tile([128, NK * 512], f16, tag="xT")
                for j in range(4):
                    r = g * 4 + j
                    xt = xpool.tile([128, H], f16, tag="x")
                    nc.sync.dma_start(out=xt[:, :], in_=x[ts(r, 128), :])
                    tp0 = tppool.tile([128, 3 * 128], f16, tag="tp0")
                    tp1 = tppool.tile([128, 3 * 128], f16, tag="tp1")
                    for k in range(3):
                        nc.tensor.transpose(tp0[:, ts(k, 128)],
                                            xt[:, ts(k, 128)], ident[:, :])
                    for k in range(3):
                        nc.tensor.transpose(tp1[:, ts(k, 128)],
                                            xt[:, ts(3 + k, 128)], ident[:, :])
                    # chunk k of row-tile j lands at col k*512 + j*128
                    o0 = xT[:, :].rearrange("p (k c) -> p k c", k=NK)
                    nc.vector.tensor_copy(
                        o0[:, 0:3, ts(j, 128)],
                        tp0[:, :].rearrange("p (k c) -> p k c", k=3))
                    nc.scalar.copy(
                        o0[:, 3:6, ts(j, 128)],
                        tp1[:, :].rearrange("p (k c) -> p k c", k=3))
                ps = mmpool.tile([L, 512], f32, tag="ps")
                for k in range(NK):
                    nc.tensor.matmul(ps[:, :], wk[k][:, :], xT[:, ts(k, 512)],
                                     start=(k == 0), stop=(k == NK - 1))
                emt = empool.tile([L, 512], f32, tag="em")
                nc.scalar.activation(emt[:, :], ps[:, :],
                                     mybir.ActivationFunctionType.Sigmoid,
                                     bias=bsb[:, :], scale=1.0)
                nc.sync.dma_start(out=em_dramT[:, ts(g, 512)], in_=emt[:, :])

            # ---- viterbi forward (unmasked), score history out ----
            # Layout: partition q = (s, cb), s in [0,16), cb in [0,8); each
            # partition owns labels c = cb*3 + j, j in [0,3). Per step:
            #   1. rhs_m[q,p] = score[q, p%3] * M0[q,p]   (own-block mask)
            #   2. scoreP[q,p] = sum_k G[k,q] rhs_m[k,p]  (PE block-diag ones:
            #      re-broadcasts each sample's full 24-score to its 8 rows)
            #   3. cand[q,(j,p)] = scoreP[q,p] + T[p, cb*3+j]
            #   4. score[q,j] = max_p cand[q,(j,p)]  -> hist
            #   5. score += em_t   (exact f32; host re-derives backpointers)
            JB = L // 8  # 3 labels per partition block
            # em16[s, p*256+tc] = em_full[s, tc+1, p]  (compact shift folded in)
            em16 = vpool.tile([BS, L * S], f32, tag="em16")
            em16_3 = em16[:, :].rearrange("s (p t) -> s p t", p=L)
            emd3 = em_dramT[:, :].rearrange("c (s t) -> s c t", s=BS)
            nc.sync.dma_start(out=em16_3[:, :, 0:S - 1], in_=emd3[:, :, 1:S])
            nc.sync.dma_start(out=emq2[:, :], in_=em16[:, :])
            e_sb = vpool.tile([BS, 128], f32, tag="e_sb")
            nc.sync.dma_start(out=e_sb[:, :], in_=emat[:, :])
            g_sb = vpool.tile([128, 128], f32, tag="g_sb")
            nc.sync.dma_start(out=g_sb[:, :], in_=gmat[:, :])
            m0_sb = vpool.tile([128, L], f32, tag="m0_sb")
            nc.sync.dma_start(out=m0_sb[:, :], in_=m0[:, :])
            cb_sb = vpool.tile([8, 128], f32, tag="cb_sb")
            nc.sync.dma_start(out=cb_sb[:, :], in_=cbsel[:, :])
            tb8_sb = vpool.tile([8, JB * L], f32, tag="tb8_sb")
            nc.sync.dma_start(out=tb8_sb[:, :], in_=tb8[:, :])
            st_sb = vpool.tile([128, JB], f32, tag="st_sb")
            nc.sync.dma_start(out=st_sb[:, :], in_=stq[:, :])
            hist_sb = vpool.tile([128, JB * S], f32, tag="hist_sb")
            rhs_m = vpool.tile([128, L], f32, tag="rhs_m")

            hist3 = hist_sb[:, :].rearrange("q (t j) -> q t j", j=JB)
            m03 = m0_sb[:, :].rearrange("q (blk jj) -> q blk jj", jj=JB)

            # hist stores PRE-emission best scores; host adds em (exact f32).
            # best_0 = start_trans; per step the PE gather accumulates
            # em_{t-1} via a second matmul against the constant sample
            # selector E (one 1.0 per column), so PSUM holds
            # score_{t-1} = gathered(best_{t-1}) + em_{t-1} exactly.
            nc.vector.tensor_copy(hist3[:, 0, :], st_sb[:, :])
            for t in range(1, S - 1):  # compact positions 1..254
                prev = (hist3[:, t - 1, :].unsqueeze(1)
                        .broadcast_to([128, 8, JB]))
                nc.vector.tensor_mul(
                    rhs_m[:, :].rearrange("q (blk jj) -> q blk jj", jj=JB),
                    prev, m03[:, :, :])
                sp = sppool.tile([128, JB * L], f32, tag="sp")
                sp3 = sp[:, :].rearrange("q (j p) -> q j p", p=L)
                # constant-input matmuls first: they depend only on
                # static tiles + the psum buffer, so the PE runs them while
                # the DVE is still on the previous step; only the
                # score-gather matmul sits on the critical path.
                nc.tensor.matmul(
                    sp3[:, :, :],
                    e_sb[:, :],
                    em16_3[:, :, t - 1].unsqueeze(1)
                    .broadcast_to([BS, JB, L]),
                    start=True, stop=False)
                nc.tensor.matmul(sp[:, :], cb_sb[:, :], tb8_sb[:, :],
                                 start=False, stop=False)
                nc.tensor.matmul(
                    sp3[:, :, :],
                    g_sb[:, :],
                    rhs_m[:, :].unsqueeze(1).broadcast_to([128, JB, L]),
                    start=False, stop=True)
                nc.vector.tensor_reduce(
                    hist3[:, t, :], sp3[:, :, :],
                    axis=mybir.AxisListType.X, op=mybir.AluOpType.max,
                )
            nc.sync.dma_start(out=hist[:, :], in_=hist_sb[:, :])
    return nc


def _run_device(x2h, W, b, T, st, trace=False):
    from concourse.bass_utils import run_bass_kernel_spmd

    if "nc" not in _DEVICE_STATE:
        nc = _build_nc()
        if not nc.is_finalized():
            nc.finalize()
        _DEVICE_STATE["nc"] = nc
    nc = _DEVICE_STATE["nc"]
    JB = L // 8
    w_in = np.ascontiguousarray(W.astype(np.float16))
    bt_in = np.ascontiguousarray(b.reshape(L, 1), np.float32)
    # q = cb*16 + s; partition q owns labels cb*3+j
    cb = np.arange(128) // BS
    g_in = (np.arange(128)[:, None] % BS == np.arange(128)[None, :] % BS
            ).astype(np.float32)
    m0_in = (np.arange(L)[None, :] // JB == cb[:, None]).astype(np.float32)
    tblk_in = np.ascontiguousarray(
        T.T[(cb[:, None] * JB + np.arange(JB)[None, :]).reshape(128, JB)]
        .reshape(128, JB * L), np.float32)
    stq_in = np.ascontiguousarray(
        st[(cb[:, None] * JB + np.arange(JB)[None, :])], np.float32)
    emat_in = (np.arange(BS)[:, None] == (np.arange(128)[None, :] % BS)
               ).astype(np.float32)
    cbsel_in = (np.arange(8)[:, None] == (np.arange(128)[None, :] // BS)
                ).astype(np.float32)
    tb8_in = np.ascontiguousarray(tblk_in[::BS], np.float32)
    in_maps = [
        {"x": x2h[c * R:(c + 1) * R], "w": w_in, "bt": bt_in,
         "gmat": g_in, "m0": m0_in, "tblk": tblk_in, "stq": stq_in,
         "emat": emat_in, "cbsel": cbsel_in, "tb8": tb8_in}
        for c in range(N_CORES)
    ]
    res = run_bass_kernel_spmd(nc, in_maps, core_ids=list(range(N_CORES)),
                               trace=trace)
    _DEVICE_STATE["last_results"] = res
    # hist holds PRE-emission best scores [128,(t,j)], q=(cb,s); emq2 holds
    # compact emissions [BS,(p,t)]. score = best + em, same f32 add as the
    # device's PSUM accumulate, so the reconstruction stays bitwise-exact.
    bests, ems = [], []
    for r in res.results:
        bests.append(r["hist"].reshape(8, BS, S, JB).transpose(1, 2, 0, 3)
                     .reshape(BS, S, L))
        ems.append(r["emq2"].reshape(BS, L, S).transpose(0, 2, 1))
    return np.concatenate(bests, axis=0), np.concatenate(ems, axis=0)


# ------------------------------------------------------------ host pieces ---

def _backtrack(best, em, tstar, T, end_trans):
    """best/em [B,S,L] f32 from device; tstar [B] last valid step.
    cand association (em + T) + best matches the device's PSUM accumulation
    order, so argmax reconstruction is bitwise-consistent."""
    ar = np.arange(B)
    final = (best[ar, tstar] + em[ar, tstar]) + end_trans[None, :]
    tag = final.argmax(1).astype(np.int64)
    path = np.empty((B, S), np.int32)
    Tf = np.ascontiguousarray(T, np.float32)
    for t in range(S - 1, 0, -1):
        path[:, t] = tag
        active = t <= tstar
        if active.any():
            cand = (em[:, t - 1, :] + Tf[:, tag].T) + best[:, t - 1, :]
            newtag = cand.argmax(1)
            tag = np.where(active, newtag, tag)
    path[:, 0] = tag
    return path


def _sigmoid(x):
    out = np.empty_like(x)
    np.negative(x, out=out)
    np.exp(out, out=out)
    out += np.float32(1.0)
    np.reciprocal(out, out=out)
    return out


def _host_full(token_features, input_mask, true_label_mask, W, b,
               transitions, start_trans, end_trans):
    """General-mask fallback, mirrors the reference exactly."""
    mask = input_mask.astype(bool)
    order = np.argsort((1 - mask).astype(np.int32), axis=1, kind="stable")
    em_full = _sigmoid(
        (token_features.reshape(-1, H) @ W + b).astype(np.float32)
    ).reshape(B, S, L)
    em = np.take_along_axis(em_full, order[:, :, None], axis=1)
    em = np.concatenate([em[:, 1:], np.zeros_like(em[:, :1])], axis=1)
    n_valid = mask.sum(axis=1)
    keep = np.arange(S)[None, :] < (n_valid[:, None] - 2)
    sb = _sigmoid(np.broadcast_to(b, (L,)).astype(np.float32))
    em = np.where(keep[:, :, None], em, sb[None, None, :])

    vmask = true_label_mask != 0
    lbl = np.arange(L)
    score = (start_trans[None, :] + em[:, 0]).astype(np.float32)
    bps = np.empty((S - 1, B, L), dtype=np.int64)
    for t in range(1, S):
        cand = score[:, :, None] + transitions[None]
        best = cand.max(axis=1) + em[:, t]
        bp = cand.argmax(axis=1)
        m = vmask[:, t][:, None]
        score = np.where(m, best, score).astype(np.float32)
        bps[t - 1] = np.where(m, bp, lbl[None, :])
    final = score + end_trans[None, :]
    tag = final.argmax(axis=1)
    path = np.empty((B, S), dtype=np.int32)
    path[:, S - 1] = tag
    for t in range(S - 2, -1, -1):
        tag = np.take_along_axis(bps[t], tag[:, None], axis=1)[:, 0]
        path[:, t] = tag
    return path


# ------------------------------------------------------------------ entry ---

def kernel(token_features, input_mask, true_label_mask, W, b,
           transitions, start_trans, end_trans):
    token_features = np.asarray(token_features, np.float32)
    input_mask = np.asarray(input_mask)
    true_label_mask = np.asarray(true_label_mask)
    W = np.asarray(W, np.float32)
    b = np.asarray(b, np.float32)
    transitions = np.asarray(transitions, np.float32)
    start_trans = np.asarray(start_trans, np.float32)
    end_trans = np.asarray(end_trans, np.float32)

    pos = np.arange(S)[None, :]
    lengths = input_mask.sum(1)
    contig = bool(
        (input_mask == (pos < lengths[:, None])).all()
        and (true_label_mask == (pos < (lengths - 2)[:, None])).all()
        and lengths.min() >= 3
    )
    if contig:
        try:
            x2h = np.ascontiguousarray(
                token_features.reshape(B * S, H).astype(np.float16))
            best, em = _run_device(x2h, W, b, transitions, start_trans,
                                   trace=_DEVICE_STATE.get("trace", False))
            tstar = (lengths - 3).astype(np.int64)
            path = _backtrack(best, em, tstar, transitions, end_trans)
            _DEVICE_STATE["used"] = True
            return path
        except Exception:
            _DEVICE_STATE["used"] = False
            import traceback
            _DEVICE_STATE["error"] = traceback.format_exc()
    else:
        _DEVICE_STATE["used"] = False
        _DEVICE_STATE["error"] = "non-contiguous masks"
    return _host_full(token_features, input_mask, true_label_mask, W, b,
                      transitions, start_trans, end_trans)
